# revision 33
# baseline (speedup 1.0000x reference)
"""ACM Graph Convolution on 8 TRN2 NeuronCores (Bass/Tile).

Strategy (dest-node sharded):
  - Each core owns N/8 destination rows.
  - Phase A: each core computes h_low/h_high = x_part @ W (bf16 TensorE),
    plus out_mlp = relu(x_part @ w_mlp) kept local.
  - AllGather h_low / h_high so each core holds the full [N, F_OUT]
    bf16 feature tables in local HBM.
  - Phase C: edges are bucketed by (dest tile of 128 rows, source window)
    on the host and padded to 128-edge chunks.  Per chunk the device
    dma_gathers the 128 source rows (4 SWDGE queues round-robin) and a
    TensorE matmul with a HOST-PRECOMPUTED one-hot*val mask accumulates
    the segment sum into PSUM: out[d,f] += sum_e mask[e,d]*h[col_e,f].
    One PSUM accumulation group per bank (start_tensor_calc zero-marks
    the whole 2KB bank).
  - relu per group into big bf16 SBUF buffers; the 3-way attention
    epilogue runs once at the end on [128, T*128] tensors.

The graph is identical on all 8 cores (SPMD): chunk capacities are the
max over cores; shorter cores run padded chunks (val=0 -> no-op).
"""

import math

import numpy as np
import ml_dtypes

CORES = 8
P = 128
TG = 4  # dest tiles (of 128 rows) per PSUM group
NQ = 4  # SWDGE queues for gather descriptor generation
FORCE_NWIN = None  # testing override for the source-window count

BF16 = ml_dtypes.bfloat16


# --------------------------------------------------------------------------
# Host-side edge preprocessing
# --------------------------------------------------------------------------

def _bucket_edges(row, col, val, n, n_per, t_tiles, n_win, half):
    core = row // n_per
    dl = row - core * n_per
    t = dl // P
    r = (dl - t * P).astype(np.int32)
    c_src = col // n_per
    r_src = col - c_src * n_per
    w = r_src // half
    wsize = np.minimum(half, n_per - w * half)
    cr = (c_src * wsize + (r_src - w * half)).astype(np.int32)
    key = (core * t_tiles + t) * n_win + w
    order = np.argsort(key, kind="stable")
    counts = np.bincount(key, minlength=CORES * t_tiles * n_win).reshape(
        CORES, t_tiles, n_win
    )
    st = np.concatenate([[0], np.cumsum(counts.reshape(-1))[:-1]])
    starts = st.reshape(CORES, t_tiles, n_win)
    return counts, starts, order, r, cr


def preprocess(x, row_low, col_low, val_low, row_high, col_high, val_high,
               w_low, w_high, w_mlp, av_low, av_high, av_mlp, att_vec):
    n, f_in = x.shape
    f_out = w_low.shape[1]
    assert n % CORES == 0
    n_per = n // CORES
    t_tiles = (n_per + P - 1) // P
    n_win = FORCE_NWIN or (1 if n <= 32000 else int(math.ceil(n / 25000.0)))
    half = int(math.ceil(n_per / n_win))
    wsizes = [min(half, n_per - w * half) for w in range(n_win)]

    groups = [list(range(i, min(i + TG, t_tiles)))
              for i in range(0, t_tiles, TG)]

    branches = []
    for (row, col, val) in ((row_low, col_low, val_low),
                            (row_high, col_high, val_high)):
        row = np.asarray(row).astype(np.int64)
        col = np.asarray(col).astype(np.int64)
        val = np.asarray(val).astype(np.float32)
        counts, starts, order, r, cr = _bucket_edges(
            row, col, val, n, n_per, t_tiles, n_win, half)
        caps = (counts.max(axis=0) + P - 1) // P  # [t_tiles, n_win]
        for t in range(t_tiles):
            if caps[t].sum() == 0:
                caps[t][0] = 1
        branches.append(dict(counts=counts, starts=starts, order=order,
                             r=r, cr=cr, val=val, caps=caps))

    # ---- global chunk schedule (identical across cores) ----
    chunk_meta = []          # cid -> (b, t, w, k)
    schedule = []            # per group: dict(tiles=[...], segs=[...])
    chunk_off = {}           # (b, t, w) -> first cid
    for g_tiles in groups:
        segs = []
        for w in range(n_win):
            for b in range(2):
                caps = branches[b]["caps"]
                start_cid = len(chunk_meta)
                items = []
                for t in g_tiles:
                    chunk_off[(b, t, w)] = len(chunk_meta)
                    for k in range(int(caps[t, w])):
                        items.append((g_tiles.index(t), len(chunk_meta), t, k))
                        chunk_meta.append((b, t, w, k))
                s_chunks = len(chunk_meta) - start_cid
                if s_chunks:
                    segs.append(dict(b=b, w=w, off=start_cid, S=s_chunks,
                                     items=items))
        schedule.append(dict(tiles=g_tiles, segs=segs))
    nchunk = len(chunk_meta)

    # one PSUM accumulation group per (group, branch) bank
    first_chunk = {}
    last_chunk = {}
    for gi, grp in enumerate(schedule):
        for seg in grp["segs"]:
            for (ti, cid, t, k) in seg["items"]:
                key = (gi, seg["b"])
                if key not in first_chunk:
                    first_chunk[key] = cid
                last_chunk[key] = cid

    # ---- per-core slot arrays ----
    gidx_maps, mask_maps = [], []
    for c in range(CORES):
        a_idx = np.zeros((nchunk, P), np.int16)
        a_r = np.zeros((nchunk, P), np.int16)
        a_v = np.zeros((nchunk, P), np.float32)
        for b in range(2):
            br = branches[b]
            for t in range(t_tiles):
                for w in range(n_win):
                    cnt = int(br["counts"][c, t, w])
                    if cnt == 0:
                        continue
                    st = int(br["starts"][c, t, w])
                    eids = br["order"][st:st + cnt]
                    # ascending source order -> near-sequential HBM reads
                    eids = eids[np.argsort(br["cr"][eids], kind="stable")]
                    off = chunk_off[(b, t, w)]
                    a_idx[off:].reshape(-1)[:cnt] = br["cr"][eids]
                    a_r[off:].reshape(-1)[:cnt] = br["r"][eids]
                    a_v[off:].reshape(-1)[:cnt] = br["val"][eids]
        gidx = a_idx.reshape(nchunk, 8, 16).transpose(2, 0, 1)\
            .reshape(16, nchunk * 8)
        gidx = np.tile(gidx, (8, 1))
        gidx_maps.append(np.ascontiguousarray(gidx))
        # one-hot * val masks: M[ci, e, d] = (a_r[ci,e]==d) * a_v[ci,e]
        m = np.zeros((nchunk, P, P), BF16)
        ci = np.arange(nchunk)[:, None]
        ei = np.arange(P)[None, :]
        m[ci, ei, a_r] = a_v.astype(BF16)
        mask_maps.append(np.ascontiguousarray(
            m.transpose(1, 0, 2).reshape(P, nchunk * P)))

    # ---- dense inputs ----
    xt = np.ascontiguousarray(np.asarray(x).astype(np.float32).T.astype(BF16))
    wcat = np.concatenate(
        [np.asarray(w).astype(np.float32) for w in (w_low, w_high, w_mlp)],
        axis=1).astype(BF16)
    avrep = np.concatenate(
        [np.tile(np.asarray(a).astype(np.float32).reshape(1, f_out), (P, 1))
         for a in (av_low, av_high, av_mlp)], axis=1).astype(BF16)

    in_maps = []
    for c in range(CORES):
        in_maps.append({
            "xt": np.ascontiguousarray(xt[:, c * n_per:(c + 1) * n_per]),
            "wcat": wcat,
            "avrep": avrep,
            "gidx": gidx_maps[c],
            "masks": mask_maps[c],
        })

    meta = dict(
        n=n, f_in=f_in, f_out=f_out, n_per=n_per, t_tiles=t_tiles,
        n_win=n_win, half=half, wsizes=wsizes, nchunk=nchunk,
        schedule=schedule, first_chunk=first_chunk, last_chunk=last_chunk,
        att=np.asarray(att_vec).astype(np.float64),
    )
    return meta, in_maps


# --------------------------------------------------------------------------
# Device graph
# --------------------------------------------------------------------------

def build_graph(meta):
    import concourse.bacc as bacc
    import concourse.tile as tile
    from concourse import mybir
    from concourse.tile_rust import add_dep_helper

    n = meta["n"]
    f_in = meta["f_in"]
    f_out = meta["f_out"]
    n_per = meta["n_per"]
    t_tiles = meta["t_tiles"]
    n_win = meta["n_win"]
    half = meta["half"]
    wsizes = meta["wsizes"]
    nchunk = meta["nchunk"]
    schedule = meta["schedule"]
    first_chunk = meta["first_chunk"]
    last_chunk = meta["last_chunk"]
    att = meta["att"]
    kc = f_in // P
    T = 3.0
    TT = t_tiles

    f32 = mybir.dt.float32
    bf16 = mybir.dt.bfloat16
    i16 = mybir.dt.int16
    AF = mybir.ActivationFunctionType
    OP = mybir.AluOpType

    nc = bacc.Bacc("TRN2", target_bir_lowering=False, debug=False,
                   num_devices=CORES, num_swdge_queues=NQ)

    xt_p = nc.declare_dram_parameter("xt", [f_in, n_per], bf16,
                                     isOutput=False)
    wcat_p = nc.declare_dram_parameter("wcat", [f_in, 3 * f_out], bf16,
                                       isOutput=False)
    avrep_p = nc.declare_dram_parameter("avrep", [P, 3 * f_out], bf16,
                                        isOutput=False)
    gidx_p = nc.declare_dram_parameter("gidx", [P, nchunk * 8], i16,
                                       isOutput=False)
    mask_p = nc.declare_dram_parameter("masks", [P, nchunk * P], bf16,
                                       isOutput=False)
    out_p = nc.declare_dram_parameter("out", [n_per, f_out], f32,
                                      isOutput=True)

    with tile.TileContext(nc) as tc:
        with tc.tile_pool(name="dram", bufs=1, space="DRAM") as dram_pool, \
             tc.tile_pool(name="static", bufs=1) as sp:
            hpart = dram_pool.tile([n_per, 2 * f_out], bf16, name="hpart")
            hall = [dram_pool.tile([CORES * wsizes[w], 2 * f_out], bf16,
                                   addr_space="Shared", name=f"hall{w}")
                    for w in range(n_win)]

            w_sb = sp.tile([P, kc, 3 * f_out], bf16, name="w_sb")
            av_sb = sp.tile([P, 3 * f_out], bf16, name="av_sb")
            gidx_sb = sp.tile([P, nchunk * 8], i16, name="gidx_sb")
            olall = sp.tile([P, TT * f_out], bf16, name="olall")
            ohall = sp.tile([P, TT * f_out], bf16, name="ohall")
            omlp = sp.tile([P, TT * f_out], bf16, name="omlp")

            nc.sync.dma_start(out=w_sb[:],
                              in_=wcat_p[:].rearrange("(k p) f -> p k f",
                                                      p=P))
            nc.sync.dma_start(out=av_sb[:], in_=avrep_p[:])
            nc.sync.dma_start(out=gidx_sb[:], in_=gidx_p[:])
            if n_per % P:
                nc.vector.memset(omlp[:], 0)

            # ---------------- Phase A: h = x @ W ----------------
            hpart_writes = []
            with tc.tile_pool(name="xtp", bufs=1) as xtp, \
                 tc.tile_pool(name="psA", bufs=2, space="PSUM") as psA, \
                 tc.tile_pool(name="hbp", bufs=3) as hbp:
                xt_sb = xtp.tile([P, kc, n_per], bf16, name="xt_sb")
                for k in range(kc):
                    nc.sync.dma_start(out=xt_sb[:, k, :],
                                      in_=xt_p[k * P:(k + 1) * P, :])
                for t in range(t_tiles):
                    m = min(P, n_per - t * P)
                    ph = psA.tile([P, 3 * f_out], f32, name="ph", tag="ph")
                    for k in range(kc):
                        nc.tensor.matmul(
                            out=ph[:m],
                            lhsT=xt_sb[:, k, t * P:t * P + m],
                            rhs=w_sb[:, k, :],
                            start=(k == 0), stop=(k == kc - 1))
                    hb = hbp.tile([P, 2 * f_out], bf16, name="hb", tag="hb")
                    nc.vector.tensor_copy(out=hb[:m], in_=ph[:m, 0:2 * f_out])
                    nc.scalar.activation(
                        out=omlp[:m, t * f_out:(t + 1) * f_out],
                        in_=ph[:m, 2 * f_out:3 * f_out], func=AF.Relu)
                    d0 = nc.sync.dma_start(out=hpart[t * P:t * P + m, :],
                                           in_=hb[:m, :])
                    hpart_writes.append((t * P, t * P + m, d0))

            # ---------- per-node-half concatenated AllGathers ----------
            cc = []
            for w in range(n_win):
                lo, hi = w * half, w * half + wsizes[w]
                cci = nc.gpsimd.collective_compute(
                    "AllGather", OP.bypass,
                    replica_groups=[list(range(CORES))],
                    ins=[hpart[lo:hi, :].opt()],
                    outs=[hall[w].opt()])
                for (r0, r1, dw) in hpart_writes:
                    if r0 < hi and r1 > lo:
                        add_dep_helper(cci.ins, dw.ins, True,
                                       reason="cc after hpart write")
                cc.append(cci)

            # -------- Phase C: gather + mask-matmul segment sum --------
            qn = 0
            with tc.tile_pool(name="gbp", bufs=5) as gbp, \
                 tc.tile_pool(name="mkp", bufs=5) as mkp, \
                 tc.tile_pool(name="psC", bufs=3, space="PSUM") as psC:
                for gi, grp in enumerate(schedule):
                    tiles = grp["tiles"]
                    tloc = len(tiles)
                    g0 = tiles[0]
                    fw = tloc * f_out
                    ps = [psC.tile([P, TG * f_out], f32, name=f"ps{b}",
                                   tag=f"ps{b}") for b in range(2)]
                    for seg in grp["segs"]:
                        b, w, off, S = seg["b"], seg["w"], seg["off"], seg["S"]
                        gb = gbp.tile([P, S * f_out], bf16, name="gb",
                                      tag="gb")
                        gat = nc.gpsimd.dma_gather(
                            out_ap=gb[:].rearrange("p (s f) -> p s f",
                                                   f=f_out),
                            in_ap=hall[w][:, b * f_out:(b + 1) * f_out],
                            idxs_ap=gidx_sb[:, off * 8:(off + S) * 8],
                            num_idxs=S * P,
                            num_idxs_reg=S * P,
                            elem_size=f_out,
                            elem_step=2 * f_out,
                            single_packet=False,
                            queue_num=qn % NQ)
                        qn += 1
                        add_dep_helper(gat.ins, cc[w].ins, True,
                                       reason="gather after allgather")
                        mk = mkp.tile([P, S * f_out], bf16, name="mk",
                                      tag="mk")
                        nc.sync.dma_start(
                            out=mk[:],
                            in_=mask_p[:, off * P:(off + S) * P])
                        for (ti, cid, t, k) in seg["items"]:
                            sl = cid - off
                            nc.tensor.matmul(
                                out=ps[b][:, ti * f_out:(ti + 1) * f_out],
                                lhsT=mk[:, sl * P:(sl + 1) * P],
                                rhs=gb[:, sl * f_out:(sl + 1) * f_out],
                                start=(cid == first_chunk[(gi, b)]),
                                stop=(cid == last_chunk[(gi, b)]),
                                skip_group_check=True)
                    nc.scalar.activation(
                        out=olall[:, g0 * f_out:g0 * f_out + fw],
                        in_=ps[0][:, :fw], func=AF.Relu)
                    nc.scalar.activation(
                        out=ohall[:, g0 * f_out:g0 * f_out + fw],
                        in_=ps[1][:, :fw], func=AF.Relu)

            # ------------- attention epilogue (two spans) -------------
            # uneven spans: bulk of the attention overlaps phase C, only a
            # small slice remains as the serial tail
            cut = max(1, (TT * 4) // 5)
            spans = [(0, cut), (cut, TT)] if TT > 1 else [(0, TT)]
            with tc.tile_pool(name="epp", bufs=1) as epp:
                for si, (s0, s1) in enumerate(spans):
                    sw = s1 - s0
                    fl, fh = s0 * f_out, s1 * f_out
                    tmp = epp.tile([P, sw * f_out], bf16, name=f"tmp{si}")
                    lg = epp.tile([P, 3 * sw], f32, name=f"lg{si}")
                    srcs = [olall[:, fl:fh], ohall[:, fl:fh], omlp[:, fl:fh]]
                    for j in range(3):
                        a3d = av_sb[:, j * f_out:(j + 1) * f_out][:, None, :] \
                            .broadcast_to([P, sw, f_out])
                        nc.vector.tensor_tensor(
                            out=tmp[:].rearrange("p (t f) -> p t f", f=f_out),
                            in0=srcs[j].rearrange("p (t f) -> p t f", f=f_out),
                            in1=a3d, op=OP.mult)
                        nc.vector.tensor_reduce(
                            out=lg[:, j * sw:(j + 1) * sw],
                            in_=tmp[:].rearrange("p (t f) -> p t f", f=f_out),
                            axis=mybir.AxisListType.X, op=OP.add)
                    sg = epp.tile([P, 3 * sw], f32, name=f"sg{si}")
                    nc.scalar.activation(out=sg[:], in_=lg[:], func=AF.Sigmoid)
                    zt = epp.tile([P, 3 * sw], f32, name=f"zt{si}")
                    t2 = epp.tile([P, sw], f32, name=f"t2{si}")
                    for j in range(3):
                        zj = zt[:, j * sw:(j + 1) * sw]
                        nc.vector.tensor_scalar(
                            out=zj, in0=sg[:, 0:sw],
                            scalar1=float(att[0, j] / T), scalar2=None,
                            op0=OP.mult)
                        for k2 in (1, 2):
                            nc.vector.tensor_scalar(
                                out=t2[:], in0=sg[:, k2 * sw:(k2 + 1) * sw],
                                scalar1=float(att[k2, j] / T), scalar2=None,
                                op0=OP.mult)
                            nc.vector.tensor_tensor(out=zj, in0=zj,
                                                    in1=t2[:], op=OP.add)
                    et = epp.tile([P, 3 * sw], f32, name=f"et{si}")
                    nc.scalar.activation(out=et[:], in_=zt[:], func=AF.Exp)
                    s3 = epp.tile([P, sw], f32, name=f"s3{si}")
                    nc.vector.tensor_tensor(out=s3[:], in0=et[:, 0:sw],
                                            in1=et[:, sw:2 * sw], op=OP.add)
                    nc.vector.tensor_tensor(out=s3[:], in0=s3[:],
                                            in1=et[:, 2 * sw:3 * sw],
                                            op=OP.add)
                    # rcp = 3/(sum e)  so that e*rcp = 3*att
                    nc.vector.reciprocal(out=s3[:], in_=s3[:])
                    nc.vector.tensor_scalar(out=s3[:], in0=s3[:], scalar1=3.0,
                                            scalar2=None, op0=OP.mult)
                    at = epp.tile([P, 3 * sw], bf16, name=f"at{si}")
                    for j in range(3):
                        nc.vector.tensor_tensor(
                            out=at[:, j * sw:(j + 1) * sw],
                            in0=et[:, j * sw:(j + 1) * sw],
                            in1=s3[:], op=OP.mult)
                    oo = epp.tile([P, sw * f_out], f32, name=f"oo{si}")
                    tmp2 = epp.tile([P, sw * f_out], f32, name=f"tmp2{si}")
                    for j in range(3):
                        dst = oo if j == 0 else tmp2
                        a3d = at[:, j * sw:(j + 1) * sw][:, :, None] \
                            .broadcast_to([P, sw, f_out])
                        nc.vector.tensor_tensor(
                            out=dst[:].rearrange("p (t f) -> p t f", f=f_out),
                            in0=srcs[j].rearrange("p (t f) -> p t f", f=f_out),
                            in1=a3d, op=OP.mult)
                        if j > 0:
                            nc.vector.tensor_tensor(out=oo[:], in0=oo[:],
                                                    in1=tmp2[:], op=OP.add)
                    # output DMA for this span
                    r0 = s0 * P
                    r1 = min(s1 * P, n_per)
                    nfull = (r1 - r0) // P
                    if nfull:
                        nc.sync.dma_start(
                            out=out_p[r0:r0 + nfull * P, :].rearrange(
                                "(t p) f -> p t f", p=P),
                            in_=oo[:, 0:nfull * f_out].rearrange(
                                "p (t f) -> p t f", f=f_out))
                    if (r1 - r0) % P:
                        m = (r1 - r0) - nfull * P
                        nc.sync.dma_start(
                            out=out_p[r0 + nfull * P:r1, :],
                            in_=oo[:m, nfull * f_out:(nfull + 1) * f_out])
    nc.compile()
    return nc


# --------------------------------------------------------------------------
# Entry point
# --------------------------------------------------------------------------

def _solve(inputs, trace=False):
    from concourse.bass_utils import run_bass_kernel_spmd

    meta, in_maps = preprocess(**inputs)
    nc = build_graph(meta)
    res = run_bass_kernel_spmd(nc, in_maps, core_ids=list(range(CORES)),
                               trace=trace)
    out = np.concatenate([res.results[c]["out"] for c in range(CORES)],
                         axis=0)
    return out.astype(np.float32), res


def kernel(**inputs):
    out, _ = _solve(inputs, trace=False)
    return out
